# revision 22
# baseline (speedup 1.0000x reference)
"""Pixel-adaptive convolution Bass/Tile kernel for Trainium2 (8 NeuronCores).

Problem (per reference):
    x      : (B=2, H=96, W=96, C=32) f32
    w_gen  : (C=32, F*K*K*F = 9216) f32
    b_gen  : (9216,) f32 (zeros in practice)
    dyn    = (x @ w_gen + b_gen).reshape(B,H,W, K*K*C, F)     # per-pixel filters
    patches= extract_patches_same(x, 3)                       # (B,H,W, K*K*C)
    out[b,h,w,f] = sum_k patches[..,k] * dyn[..,k,f]

Algebraic restructuring used here (per pixel p, k=(ki,kj,c), channels c'):
    out[p,f] = sum_{c'} x[p,c'] * M2[(f,c'), p]
    M2[(f,c'), p] = sum_k W4[k, (f,c')] * patches[p,k],   W4[k, f*32+c'] = w_gen[c', k*32+f]

So per core (data-parallel over 24 (b,h)-rows each):
    1. patchesT[k, p] materialized as 9 shifted SBUF copies of the transposed,
       zero-padded input slab (no gather needed: patchesT for shift (ki,kj) is
       just a shifted window of xT_pad).
    2. PE: M2T = W4^T @ patchesT per 128-wide (f,c') tile  (3 K-chunks of 128).
    3. DVE: prodT = M2T * xTrep (x^T replicated 4x over partitions).
    4. PE: out^T[f, p] = sum over (f,c') partitions via 0/1 selection matmuls
       (contract c' groups), accumulated in PSUM.
    5. out^T DMA'd out; host transposes + reassembles.

The bias (b_gen) contribution sum_k patches[p,k]*Bg[k,f] is added with 3 extra
accumulating matmuls only when b_gen is nonzero (it is structurally zero here).
"""

import numpy as np

import concourse.bass as bass
import concourse.mybir as mybir
import concourse.tile as tile
from concourse import bacc
from concourse.bass import ts

F32 = mybir.dt.float32
F32R = mybir.dt.float32r  # relaxed fp32: full-rate PE matmul (fp32 is 4x slower)

B, H, W, C = 2, 96, 96, 32
KS = 3  # conv kernel size
NCORES = 8
RPC = (B * H) // NCORES          # (b,h)-rows per core = 24
PIX = RPC * W                    # real pixels per core = 2304
PADW = W + 2                     # 98
XTCOLS = (RPC + 2) * PADW        # 26*98 = 2548
NSHIFT = KS * KS                 # 9
KDIM = NSHIFT * C                # 288
NFC = C * C                      # 1024 (f,c') pairs
NPASS = 512                      # pixels per matmul pass (PSUM bank / fp32 max)
LAG = 4                          # sel-matmul emission lag (hide DVE latency)


def _build_nc(has_bias: bool, reps: int = 1, mm_dt=F32R) -> bass.Bass:
    """Build the Bass module.  reps>1 wraps the whole body (input DMAs,
    patchesT build, compute, output DMAs) in a hardware For_i loop — used
    only for benchmarking (amortizes the ~80ms axon dispatch overhead)."""
    nc = bacc.Bacc("TRN2", target_bir_lowering=False, debug=False)

    xt_d = nc.dram_tensor("xt", [C, XTCOLS], mm_dt, kind="ExternalInput")
    xr_d = nc.dram_tensor("xr", [128, PIX], F32, kind="ExternalInput")
    w4_d = nc.dram_tensor("w4", [128, 2, NFC], mm_dt, kind="ExternalInput")
    w4b_d = nc.dram_tensor("w4b", [128, NFC], mm_dt, kind="ExternalInput")
    sel_d = nc.dram_tensor("sel", [128, NCORES, C], mm_dt, kind="ExternalInput")
    if has_bias:
        bg_d = nc.dram_tensor("bg", [128, 3, C], mm_dt, kind="ExternalInput")
    out_d = nc.dram_tensor("outT", [C, PIX], F32, kind="ExternalOutput")

    passes = []
    p0 = 0
    while p0 < PIX:
        n = min(NPASS, PIX - p0)
        passes.append((p0, n))
        p0 += n

    with tile.TileContext(nc) as tc:
        with (
            tc.tile_pool(name="const", bufs=1) as const_pool,
            tc.tile_pool(name="prod", bufs=5) as prod_pool,
            tc.tile_pool(name="outsb", bufs=2) as out_pool,
            tc.tile_pool(name="psum_m2", bufs=5, space="PSUM") as psum_m2,
            tc.tile_pool(name="psum_out", bufs=2, space="PSUM") as psum_out,
        ):

          def body():
            xt_sb = const_pool.tile([C, XTCOLS], mm_dt, tag="xt")
            w4_sb = const_pool.tile([128, 2, NFC], mm_dt, tag="w4")
            w4b_sb = const_pool.tile([128, NFC], mm_dt, tag="w4b")
            sel_sb = const_pool.tile([128, NCORES, C], mm_dt, tag="sel")
            patchesT = const_pool.tile([128, 3, PIX], mm_dt, tag="patchesT")
            xtrep = const_pool.tile([128, PIX], F32, tag="xtrep")
            if has_bias:
                bg_sb = const_pool.tile([128, 3, C], mm_dt, tag="bg")
                nc.sync.dma_start(bg_sb[:], bg_d[:])

            nc.sync.dma_start(xt_sb[:], xt_d[:])
            nc.sync.dma_start(xtrep[:], xr_d[:])
            nc.sync.dma_start(w4_sb[:], w4_d[:])
            nc.sync.dma_start(w4b_sb[:], w4b_d[:])
            nc.sync.dma_start(sel_sb[:], sel_d[:])

            # padded-slab view [c, row, col] of the transposed input
            xtv = xt_sb[:].rearrange("p (r w) -> p r w", w=PADW)

            # shifted copies build patchesT: shifts 0-7 fill chunks 0-1;
            # shift 8 (k=256..287) is replicated at all 4 partition bases of
            # chunk 2 so its K=32 matmuls can be row-tiled 4-concurrent.
            for s in range(NSHIFT):
                ki, kj = divmod(s, KS)
                chunk, prow = divmod(s, 4)
                reps_dst = [prow] if s < 8 else [0, 1, 2, 3]
                for pr in reps_dst:
                    dst = patchesT[32 * pr : 32 * pr + 32, chunk, :].rearrange(
                        "p (r w) -> p r w", w=W
                    )
                    nc.sync.dma_start(dst, xtv[:, ki : ki + RPC, kj : kj + W])

            pending = []

            def flush_one():
                t, prodT, out_ps, n, px0 = pending.pop(0)
                if t == 0:
                    if has_bias:
                        for c in range(3):
                            nc.tensor.matmul(
                                out_ps[:, :n],
                                bg_sb[:, c, :],
                                patchesT[:, c, px0 : px0 + n],
                                start=(c == 0),
                                stop=False,
                            )
                nc.tensor.matmul(
                    out_ps[:, :n],
                    sel_sb[:, t, :],
                    prodT[:, :n],
                    start=(t == 0 and not has_bias),
                    stop=(t == NCORES - 1),
                )
                if t == NCORES - 1:
                    o_sb = out_pool.tile([C, NPASS], F32, tag="osb")
                    nc.scalar.copy(o_sb[:, :n], out_ps[:, :n])
                    nc.sync.dma_start(out_d[:, px0 : px0 + n], o_sb[:, :n])

            for px0, n in passes:
                out_ps = psum_out.tile([C, NPASS], F32, tag="outps")
                for g in range(2):  # two groups of 4 (f,c')-tiles
                    m2ts = []
                    for u in range(4):
                        t = 4 * g + u
                        m2t = psum_m2.tile([128, NPASS], F32, tag="m2t")
                        m2ts.append(m2t)
                        for c in range(2):
                            nc.tensor.matmul(
                                m2t[:, :n],
                                w4_sb[:, c, ts(t, 128)],
                                patchesT[:, c, px0 : px0 + n],
                                start=(c == 0),
                                stop=False,
                            )
                    # 4 concurrent K=32 row-tiled matmuls finish the 4 tiles
                    for u in range(4):
                        t = 4 * g + u
                        nc.tensor.matmul(
                            m2ts[u][:, :n],
                            w4b_sb[ts(u, 32), ts(t, 128)],
                            patchesT[ts(u, 32), 2, px0 : px0 + n],
                            start=False,
                            stop=True,
                            tile_position=(32 * u, 0),
                        )
                    for u in range(4):
                        t = 4 * g + u
                        prodT = prod_pool.tile([128, NPASS], mm_dt, tag="prodT")
                        nc.vector.tensor_tensor(
                            prodT[:, :n],
                            m2ts[u][:, :n],
                            xtrep[:, px0 : px0 + n],
                            op=mybir.AluOpType.mult,
                        )
                        pending.append((t, prodT, out_ps, n, px0))
                    while len(pending) > LAG:
                        flush_one()
            while pending:
                flush_one()

          if reps == 1:
              body()
          else:
              with tc.For_i(0, reps, 1):
                  body()

    nc.compile()
    return nc


def _host_prep(x, w_gen, b_gen):
    """Build per-core input maps (host-side sharding / layout only)."""
    x = np.ascontiguousarray(x, dtype=np.float32)
    w_gen = np.ascontiguousarray(w_gen, dtype=np.float32)
    b_gen = np.ascontiguousarray(b_gen, dtype=np.float32)

    has_bias = bool(np.any(b_gen))

    # W4[k, f*32+c'] = w_gen[c', k*32+f]; pad K=288 into 3 chunks of 128
    w4 = np.transpose(w_gen.reshape(C, KDIM, C), (1, 2, 0)).reshape(KDIM, NFC)
    w4p = np.zeros((128, 2, NFC), np.float32)
    w4p[:, 0, :] = w4[0:128]
    w4p[:, 1, :] = w4[128:256]
    # chunk-2 weights (k=256..287) at partition base 32*(t%4) for row tiling
    w4b = np.zeros((128, NFC), np.float32)
    for t in range(NCORES):
        u = t % 4
        w4b[32 * u : 32 * u + 32, 128 * t : 128 * t + 128] = w4[256:KDIM, 128 * t : 128 * t + 128]

    # selection matrices: sel[p, t, f] = 1 iff f == 4t + p//32
    sel = np.zeros((128, NCORES, C), np.float32)
    for t in range(NCORES):
        for p in range(128):
            sel[p, t, 4 * t + p // 32] = 1.0

    if has_bias:
        bg = b_gen.reshape(KDIM, C)
        bgp = np.zeros((128, 3, C), np.float32)
        bgp[:, 0, :] = bg[0:128]
        bgp[:, 1, :] = bg[128:256]
        bgp[:C, 2, :] = bg[256:KDIM]

    xpad = np.zeros((B, H + 2, W + 2, C), np.float32)
    xpad[:, 1 : H + 1, 1 : W + 1, :] = x

    in_maps = []
    for core in range(NCORES):
        b, j = divmod(core, NCORES // B)
        slab = xpad[b, RPC * j : RPC * j + RPC + 2]           # (26, 98, 32)
        xt = np.ascontiguousarray(slab.reshape(XTCOLS, C).T)  # (32, 2548)
        # x^T at real pixels, replicated 4x over partitions (TT multiplier)
        xc = x[b, RPC * j : RPC * j + RPC].reshape(PIX, C).T  # (32, 2304)
        xr = np.ascontiguousarray(np.tile(xc, (4, 1)))        # (128, 2304)
        m = {"xt": xt, "xr": xr, "w4": w4p, "w4b": w4b, "sel": sel}
        if has_bias:
            m["bg"] = bgp
        in_maps.append(m)
    return in_maps, has_bias


def _assemble(outs):
    out = np.empty((B, H, W, C), np.float32)
    for core in range(NCORES):
        b, j = divmod(core, NCORES // B)
        out[b, RPC * j : RPC * j + RPC] = (
            outs[core]["outT"].T.reshape(RPC, W, C)
        )
    return out


# ---------------------------------------------------------------------------
# Execution: persistent jitted shard_map callable (axon/PJRT path), so that
# repeated kernel() calls don't re-trace.  Mirrors bass2jax.run_bass_via_pjrt.
# ---------------------------------------------------------------------------
_RUNNERS = {}


def _get_runner(has_bias: bool, reps: int = 1):
    key = (has_bias, reps)
    if key in _RUNNERS:
        return _RUNNERS[key]

    import jax
    from jax.experimental.shard_map import shard_map
    from jax.sharding import Mesh, NamedSharding, PartitionSpec

    from concourse import bass2jax

    nc = _build_nc(has_bias, reps)
    bass2jax.install_neuronx_cc_hook()

    partition_name = (
        nc.partition_id_tensor.name if nc.partition_id_tensor else None
    )
    in_names, out_names, out_avals, zero_shapes = [], [], [], []
    for alloc in nc.m.functions[0].allocations:
        if not isinstance(alloc, mybir.MemoryLocationSet):
            continue
        name = alloc.memorylocations[0].name
        if alloc.kind == "ExternalInput":
            if name != partition_name:
                in_names.append(name)
        elif alloc.kind == "ExternalOutput":
            out_names.append(name)
            shape = tuple(alloc.tensor_shape)
            dtype = mybir.dt.np(alloc.dtype)
            out_avals.append(jax.core.ShapedArray(shape, dtype))
            zero_shapes.append((shape, dtype))
    n_params = len(in_names)
    n_outs = len(out_names)
    all_in_names = in_names + out_names
    if partition_name is not None:
        all_in_names = all_in_names + [partition_name]

    def _body(*args):
        operands = list(args)
        if partition_name is not None:
            operands.append(bass2jax.partition_id_tensor())
        outs = bass2jax._bass_exec_p.bind(
            *operands,
            out_avals=tuple(out_avals),
            in_names=tuple(all_in_names),
            out_names=tuple(out_names),
            lowering_input_output_aliases=(),
            sim_require_finite=True,
            sim_require_nnan=True,
            nc=nc,
        )
        return tuple(outs)

    devices = jax.devices()[:NCORES]
    assert len(devices) == NCORES, f"need {NCORES} cores, got {len(devices)}"
    mesh = Mesh(np.asarray(devices), ("core",))
    donate = tuple(range(n_params, n_params + n_outs))
    sharded = jax.jit(
        shard_map(
            _body,
            mesh=mesh,
            in_specs=(PartitionSpec("core"),) * (n_params + n_outs),
            out_specs=(PartitionSpec("core"),) * n_outs,
            check_rep=False,
        ),
        donate_argnums=donate,
        keep_unused=True,
    )

    def run(in_maps):
        concat_in = [
            np.concatenate([np.asarray(m[name]) for m in in_maps], axis=0)
            for name in in_names
        ]
        concat_zeros = [
            np.zeros((NCORES * s[0], *s[1:]), dt) for (s, dt) in zero_shapes
        ]
        out_arrs = sharded(*concat_in, *concat_zeros)
        out_arrs = [np.asarray(a) for a in jax.block_until_ready(out_arrs)]
        return [
            {
                name: out_arrs[i].reshape(NCORES, *out_avals[i].shape)[c]
                for i, name in enumerate(out_names)
            }
            for c in range(NCORES)
        ]

    def time_resident(in_maps, n_calls=12):
        """Min wall time per dispatch with device-resident operands and no
        output download (measures dispatch overhead + on-device exec)."""
        import time as _time

        sh = NamedSharding(mesh, PartitionSpec("core"))
        concat_in = [
            jax.device_put(
                np.concatenate([np.asarray(m[name]) for m in in_maps], axis=0), sh
            )
            for name in in_names
        ]
        concat_zeros = [
            jax.device_put(np.zeros((NCORES * s[0], *s[1:]), dt), sh)
            for (s, dt) in zero_shapes
        ]
        jax.block_until_ready(concat_in)
        jax.block_until_ready(concat_zeros)
        nodonate = jax.jit(
            shard_map(
                _body,
                mesh=mesh,
                in_specs=(PartitionSpec("core"),) * (n_params + n_outs),
                out_specs=(PartitionSpec("core"),) * n_outs,
                check_rep=False,
            ),
            keep_unused=True,
        )
        jax.block_until_ready(nodonate(*concat_in, *concat_zeros))  # compile
        best = float("inf")
        for _ in range(n_calls):
            t0 = _time.perf_counter()
            jax.block_until_ready(nodonate(*concat_in, *concat_zeros))
            best = min(best, _time.perf_counter() - t0)
        return best

    run.time_resident = time_resident
    _RUNNERS[key] = run
    return run


def kernel(x, w_gen, b_gen):
    in_maps, has_bias = _host_prep(x, w_gen, b_gen)
    run = _get_runner(has_bias)
    outs = run(in_maps)
    return _assemble(outs)


BENCH_REPS = 64


def benchmark(x, w_gen, b_gen, iters=12):
    """Estimate per-execution device time: run the kernel body BENCH_REPS
    times inside one NEFF (hardware For_i around the whole body) and take the
    difference quotient against the single-rep NEFF.  This cancels the ~80ms
    axon dispatch overhead and the NEFF load/launch constants."""
    in_maps, has_bias = _host_prep(x, w_gen, b_gen)
    run1 = _get_runner(has_bias, 1)
    runN = _get_runner(has_bias, BENCH_REPS)
    t1 = run1.time_resident(in_maps, iters)
    tn = runN.time_resident(in_maps, iters)
    per_exec = (tn - t1) / (BENCH_REPS - 1)
    print(
        f"[benchmark] plain={t1 * 1e3:.2f} ms  loop{BENCH_REPS}={tn * 1e3:.2f} ms"
        f"  -> per-exec {per_exec * 1e6:.1f} us"
    )
    return per_exec * 1e9


# revision 25
# speedup vs baseline: 7.6891x; 7.6891x over previous
"""Pixel-adaptive convolution Bass/Tile kernel for Trainium2 (8 NeuronCores).

Problem (per reference):
    x      : (B=2, H=96, W=96, C=32) f32
    w_gen  : (C=32, F*K*K*F = 9216) f32
    b_gen  : (9216,) f32 (zeros in practice)
    dyn    = (x @ w_gen + b_gen).reshape(B,H,W, K*K*C, F)     # per-pixel filters
    patches= extract_patches_same(x, 3)                       # (B,H,W, K*K*C)
    out[b,h,w,f] = sum_k patches[..,k] * dyn[..,k,f]

Algebraic restructuring used here (per pixel p, k=(ki,kj,c), channels c'):
    out[p,f] = sum_{c'} x[p,c'] * M2[(f,c'), p]
    M2[(f,c'), p] = sum_k W4[k, (f,c')] * patches[p,k],   W4[k, f*32+c'] = w_gen[c', k*32+f]

So per core (data-parallel over 24 (b,h)-rows each):
    1. patchesT[k, p] materialized as 9 shifted SBUF copies of the transposed,
       zero-padded input slab (no gather needed: patchesT for shift (ki,kj) is
       just a shifted window of xT_pad).
    2. PE: M2T = W4^T @ patchesT per 128-wide (f,c') tile  (3 K-chunks of 128).
    3. DVE: prodT = M2T * xTrep (x^T replicated 4x over partitions).
    4. PE: out^T[f, p] = sum over (f,c') partitions via 0/1 selection matmuls
       (contract c' groups), accumulated in PSUM.
    5. out^T DMA'd out; host transposes + reassembles.

The bias (b_gen) contribution sum_k patches[p,k]*Bg[k,f] is added with 3 extra
accumulating matmuls only when b_gen is nonzero (it is structurally zero here).
"""

import numpy as np

import concourse.bass as bass
import concourse.mybir as mybir
import concourse.tile as tile
from concourse import bacc
from concourse.bass import ts

F32 = mybir.dt.float32
F32R = mybir.dt.float32r  # relaxed fp32: full-rate PE matmul (fp32 is 4x slower)

B, H, W, C = 2, 96, 96, 32
KS = 3  # conv kernel size
NCORES = 8
RPC = (B * H) // NCORES          # (b,h)-rows per core = 24
PIX = RPC * W                    # real pixels per core = 2304
PADW = W + 2                     # 98
XTCOLS = (RPC + 2) * PADW        # 26*98 = 2548
NSHIFT = KS * KS                 # 9
KDIM = NSHIFT * C                # 288
NFC = C * C                      # 1024 (f,c') pairs
NPASS = 512                      # pixels per matmul pass (PSUM bank / fp32 max)
LAG = 4                          # sel-matmul emission lag (hide DVE latency)


def _build_nc(has_bias: bool, reps: int = 1, mm_dt=F32R, pack_c2: bool | None = None) -> bass.Bass:
    """Build the Bass module.  reps>1 wraps the whole body (input DMAs,
    patchesT build, compute, output DMAs) in a hardware For_i loop — used
    only for benchmarking (amortizes the ~80ms axon dispatch overhead)."""
    if pack_c2 is None:
        import os

        pack_c2 = os.environ.get("KERNEL_PACK_C2", "1") == "1"
    nc = bacc.Bacc("TRN2", target_bir_lowering=False, debug=False)

    xt_d = nc.dram_tensor("xt", [C, XTCOLS], mm_dt, kind="ExternalInput")
    xr_d = nc.dram_tensor("xr", [128, PIX], F32, kind="ExternalInput")
    w4_d = nc.dram_tensor("w4", [128, 2, NFC], mm_dt, kind="ExternalInput")
    w4b_d = nc.dram_tensor("w4b", [128, NFC], mm_dt, kind="ExternalInput")
    sel_d = nc.dram_tensor("sel", [128, NCORES, C], mm_dt, kind="ExternalInput")
    if has_bias:
        bg_d = nc.dram_tensor("bg", [128, 3, C], mm_dt, kind="ExternalInput")
    out_d = nc.dram_tensor("outT", [C, PIX], F32, kind="ExternalOutput")

    passes = []
    p0 = 0
    while p0 < PIX:
        n = min(NPASS, PIX - p0)
        passes.append((p0, n))
        p0 += n

    with tile.TileContext(nc) as tc:
        with (
            tc.tile_pool(name="const", bufs=1) as const_pool,
            tc.tile_pool(name="prod", bufs=5) as prod_pool,
            tc.tile_pool(name="outsb", bufs=2) as out_pool,
            tc.tile_pool(name="psum_m2", bufs=5, space="PSUM") as psum_m2,
            tc.tile_pool(name="psum_out", bufs=2, space="PSUM") as psum_out,
        ):

          def body():
            xt_sb = const_pool.tile([C, XTCOLS], mm_dt, tag="xt")
            w4_sb = const_pool.tile([128, 2, NFC], mm_dt, tag="w4")
            w4b_sb = const_pool.tile([128, NFC], mm_dt, tag="w4b")
            sel_sb = const_pool.tile([128, NCORES, C], mm_dt, tag="sel")
            patchesT = const_pool.tile([128, 3, PIX], mm_dt, tag="patchesT")
            xtrep = const_pool.tile([128, PIX], F32, tag="xtrep")
            if has_bias:
                bg_sb = const_pool.tile([128, 3, C], mm_dt, tag="bg")
                nc.sync.dma_start(bg_sb[:], bg_d[:])

            nc.sync.dma_start(xt_sb[:], xt_d[:])
            nc.sync.dma_start(xtrep[:], xr_d[:])
            nc.sync.dma_start(w4_sb[:], w4_d[:])
            nc.sync.dma_start(w4b_sb[:], w4b_d[:])
            nc.sync.dma_start(sel_sb[:], sel_d[:])

            # padded-slab view [c, row, col] of the transposed input
            xtv = xt_sb[:].rearrange("p (r w) -> p r w", w=PADW)

            # shifted copies build patchesT: shifts 0-7 fill chunks 0-1;
            # shift 8 (k=256..287) is replicated at all 4 partition bases of
            # chunk 2 so its K=32 matmuls can be row-tiled 4-concurrent.
            for s in range(NSHIFT):
                ki, kj = divmod(s, KS)
                chunk, prow = divmod(s, 4)
                reps_dst = [prow] if s < 8 else [0, 1, 2, 3]
                for pr in reps_dst:
                    dst = patchesT[32 * pr : 32 * pr + 32, chunk, :].rearrange(
                        "p (r w) -> p r w", w=W
                    )
                    nc.sync.dma_start(dst, xtv[:, ki : ki + RPC, kj : kj + W])

            pending = []

            def flush_one():
                t, prodT, out_ps, n, px0 = pending.pop(0)
                if t == 0:
                    if has_bias:
                        for c in range(3):
                            nc.tensor.matmul(
                                out_ps[:, :n],
                                bg_sb[:, c, :],
                                patchesT[:, c, px0 : px0 + n],
                                start=(c == 0),
                                stop=False,
                            )
                nc.tensor.matmul(
                    out_ps[:, :n],
                    sel_sb[:, t, :],
                    prodT[:, :n],
                    start=(t == 0 and not has_bias),
                    stop=(t == NCORES - 1),
                )
                if t == NCORES - 1:
                    o_sb = out_pool.tile([C, NPASS], F32, tag="osb")
                    nc.scalar.copy(o_sb[:, :n], out_ps[:, :n])
                    nc.sync.dma_start(out_d[:, px0 : px0 + n], o_sb[:, :n])

            for px0, n in passes:
                out_ps = psum_out.tile([C, NPASS], F32, tag="outps")
                for g in range(2):  # two groups of 4 (f,c')-tiles
                    m2ts = []
                    for u in range(4):
                        t = 4 * g + u
                        m2t = psum_m2.tile([128, NPASS], F32, tag="m2t")
                        m2ts.append(m2t)
                        for c in range(2):
                            nc.tensor.matmul(
                                m2t[:, :n],
                                w4_sb[:, c, ts(t, 128)],
                                patchesT[:, c, px0 : px0 + n],
                                start=(c == 0),
                                stop=False,
                            )
                    # 4 concurrent K=32 row-tiled matmuls finish the 4 tiles
                    for u in range(4):
                        t = 4 * g + u
                        up = u if pack_c2 else 0
                        kw = {"tile_position": (32 * u, 0)} if pack_c2 else {}
                        nc.tensor.matmul(
                            m2ts[u][:, :n],
                            w4b_sb[ts(up, 32), ts(t, 128)],
                            patchesT[ts(up, 32), 2, px0 : px0 + n],
                            start=False,
                            stop=True,
                            **kw,
                        )
                    for u in range(4):
                        t = 4 * g + u
                        prodT = prod_pool.tile([128, NPASS], mm_dt, tag="prodT")
                        nc.vector.tensor_tensor(
                            prodT[:, :n],
                            m2ts[u][:, :n],
                            xtrep[:, px0 : px0 + n],
                            op=mybir.AluOpType.mult,
                        )
                        pending.append((t, prodT, out_ps, n, px0))
                    while len(pending) > LAG:
                        flush_one()
            while pending:
                flush_one()

          if reps == 1:
              body()
          else:
              with tc.For_i(0, reps, 1):
                  body()

    nc.compile()
    return nc


def _host_prep(x, w_gen, b_gen):
    """Build per-core input maps (host-side sharding / layout only)."""
    x = np.ascontiguousarray(x, dtype=np.float32)
    w_gen = np.ascontiguousarray(w_gen, dtype=np.float32)
    b_gen = np.ascontiguousarray(b_gen, dtype=np.float32)

    has_bias = bool(np.any(b_gen))

    # W4[k, f*32+c'] = w_gen[c', k*32+f]; pad K=288 into 3 chunks of 128
    w4 = np.transpose(w_gen.reshape(C, KDIM, C), (1, 2, 0)).reshape(KDIM, NFC)
    w4p = np.zeros((128, 2, NFC), np.float32)
    w4p[:, 0, :] = w4[0:128]
    w4p[:, 1, :] = w4[128:256]
    # chunk-2 weights (k=256..287) at partition base 32*(t%4) for row tiling
    w4b = np.zeros((128, NFC), np.float32)
    for t in range(NCORES):
        u = t % 4
        w4b[32 * u : 32 * u + 32, 128 * t : 128 * t + 128] = w4[256:KDIM, 128 * t : 128 * t + 128]

    # selection matrices: sel[p, t, f] = 1 iff f == 4t + p//32
    sel = np.zeros((128, NCORES, C), np.float32)
    for t in range(NCORES):
        for p in range(128):
            sel[p, t, 4 * t + p // 32] = 1.0

    if has_bias:
        bg = b_gen.reshape(KDIM, C)
        bgp = np.zeros((128, 3, C), np.float32)
        bgp[:, 0, :] = bg[0:128]
        bgp[:, 1, :] = bg[128:256]
        bgp[:C, 2, :] = bg[256:KDIM]

    xpad = np.zeros((B, H + 2, W + 2, C), np.float32)
    xpad[:, 1 : H + 1, 1 : W + 1, :] = x

    in_maps = []
    for core in range(NCORES):
        b, j = divmod(core, NCORES // B)
        slab = xpad[b, RPC * j : RPC * j + RPC + 2]           # (26, 98, 32)
        xt = np.ascontiguousarray(slab.reshape(XTCOLS, C).T)  # (32, 2548)
        # x^T at real pixels, replicated 4x over partitions (TT multiplier)
        xc = x[b, RPC * j : RPC * j + RPC].reshape(PIX, C).T  # (32, 2304)
        xr = np.ascontiguousarray(np.tile(xc, (4, 1)))        # (128, 2304)
        m = {"xt": xt, "xr": xr, "w4": w4p, "w4b": w4b, "sel": sel}
        if has_bias:
            m["bg"] = bgp
        in_maps.append(m)
    return in_maps, has_bias


def _assemble(outs):
    out = np.empty((B, H, W, C), np.float32)
    for core in range(NCORES):
        b, j = divmod(core, NCORES // B)
        out[b, RPC * j : RPC * j + RPC] = (
            outs[core]["outT"].T.reshape(RPC, W, C)
        )
    return out


# ---------------------------------------------------------------------------
# Execution: persistent jitted shard_map callable (axon/PJRT path), so that
# repeated kernel() calls don't re-trace.  Mirrors bass2jax.run_bass_via_pjrt.
# ---------------------------------------------------------------------------
_RUNNERS = {}


def _get_runner(has_bias: bool, reps: int = 1):
    key = (has_bias, reps)
    if key in _RUNNERS:
        return _RUNNERS[key]

    import jax
    from jax.experimental.shard_map import shard_map
    from jax.sharding import Mesh, NamedSharding, PartitionSpec

    from concourse import bass2jax

    nc = _build_nc(has_bias, reps)
    bass2jax.install_neuronx_cc_hook()

    partition_name = (
        nc.partition_id_tensor.name if nc.partition_id_tensor else None
    )
    in_names, out_names, out_avals, zero_shapes = [], [], [], []
    for alloc in nc.m.functions[0].allocations:
        if not isinstance(alloc, mybir.MemoryLocationSet):
            continue
        name = alloc.memorylocations[0].name
        if alloc.kind == "ExternalInput":
            if name != partition_name:
                in_names.append(name)
        elif alloc.kind == "ExternalOutput":
            out_names.append(name)
            shape = tuple(alloc.tensor_shape)
            dtype = mybir.dt.np(alloc.dtype)
            out_avals.append(jax.core.ShapedArray(shape, dtype))
            zero_shapes.append((shape, dtype))
    n_params = len(in_names)
    n_outs = len(out_names)
    all_in_names = in_names + out_names
    if partition_name is not None:
        all_in_names = all_in_names + [partition_name]

    def _body(*args):
        operands = list(args)
        if partition_name is not None:
            operands.append(bass2jax.partition_id_tensor())
        outs = bass2jax._bass_exec_p.bind(
            *operands,
            out_avals=tuple(out_avals),
            in_names=tuple(all_in_names),
            out_names=tuple(out_names),
            lowering_input_output_aliases=(),
            sim_require_finite=True,
            sim_require_nnan=True,
            nc=nc,
        )
        return tuple(outs)

    devices = jax.devices()[:NCORES]
    assert len(devices) == NCORES, f"need {NCORES} cores, got {len(devices)}"
    mesh = Mesh(np.asarray(devices), ("core",))
    donate = tuple(range(n_params, n_params + n_outs))
    sharded = jax.jit(
        shard_map(
            _body,
            mesh=mesh,
            in_specs=(PartitionSpec("core"),) * (n_params + n_outs),
            out_specs=(PartitionSpec("core"),) * n_outs,
            check_rep=False,
        ),
        donate_argnums=donate,
        keep_unused=True,
    )

    def run(in_maps):
        concat_in = [
            np.concatenate([np.asarray(m[name]) for m in in_maps], axis=0)
            for name in in_names
        ]
        concat_zeros = [
            np.zeros((NCORES * s[0], *s[1:]), dt) for (s, dt) in zero_shapes
        ]
        out_arrs = sharded(*concat_in, *concat_zeros)
        out_arrs = [np.asarray(a) for a in jax.block_until_ready(out_arrs)]
        return [
            {
                name: out_arrs[i].reshape(NCORES, *out_avals[i].shape)[c]
                for i, name in enumerate(out_names)
            }
            for c in range(NCORES)
        ]

    def time_resident(in_maps, n_calls=12):
        """Min wall time per dispatch with device-resident operands and no
        output download (measures dispatch overhead + on-device exec)."""
        import time as _time

        sh = NamedSharding(mesh, PartitionSpec("core"))
        concat_in = [
            jax.device_put(
                np.concatenate([np.asarray(m[name]) for m in in_maps], axis=0), sh
            )
            for name in in_names
        ]
        concat_zeros = [
            jax.device_put(np.zeros((NCORES * s[0], *s[1:]), dt), sh)
            for (s, dt) in zero_shapes
        ]
        jax.block_until_ready(concat_in)
        jax.block_until_ready(concat_zeros)
        nodonate = jax.jit(
            shard_map(
                _body,
                mesh=mesh,
                in_specs=(PartitionSpec("core"),) * (n_params + n_outs),
                out_specs=(PartitionSpec("core"),) * n_outs,
                check_rep=False,
            ),
            keep_unused=True,
        )
        jax.block_until_ready(nodonate(*concat_in, *concat_zeros))  # compile
        best = float("inf")
        for _ in range(n_calls):
            t0 = _time.perf_counter()
            jax.block_until_ready(nodonate(*concat_in, *concat_zeros))
            best = min(best, _time.perf_counter() - t0)
        return best

    run.time_resident = time_resident
    _RUNNERS[key] = run
    return run


def kernel(x, w_gen, b_gen):
    in_maps, has_bias = _host_prep(x, w_gen, b_gen)
    run = _get_runner(has_bias)
    outs = run(in_maps)
    return _assemble(outs)


BENCH_REPS = 64


def benchmark(x, w_gen, b_gen, iters=12):
    """Estimate per-execution device time: run the kernel body BENCH_REPS
    times inside one NEFF (hardware For_i around the whole body) and take the
    difference quotient against the single-rep NEFF.  This cancels the ~80ms
    axon dispatch overhead and the NEFF load/launch constants."""
    in_maps, has_bias = _host_prep(x, w_gen, b_gen)
    run1 = _get_runner(has_bias, 1)
    runN = _get_runner(has_bias, BENCH_REPS)
    t1 = run1.time_resident(in_maps, iters)
    tn = runN.time_resident(in_maps, iters)
    per_exec = (tn - t1) / (BENCH_REPS - 1)
    print(
        f"[benchmark] plain={t1 * 1e3:.2f} ms  loop{BENCH_REPS}={tn * 1e3:.2f} ms"
        f"  -> per-exec {per_exec * 1e6:.1f} us"
    )
    return per_exec * 1e9


# revision 26
# speedup vs baseline: 8.2838x; 1.0773x over previous
"""Pixel-adaptive convolution Bass/Tile kernel for Trainium2 (8 NeuronCores).

Problem (per reference):
    x      : (B=2, H=96, W=96, C=32) f32
    w_gen  : (C=32, F*K*K*F = 9216) f32
    b_gen  : (9216,) f32 (zeros in practice)
    dyn    = (x @ w_gen + b_gen).reshape(B,H,W, K*K*C, F)     # per-pixel filters
    patches= extract_patches_same(x, 3)                       # (B,H,W, K*K*C)
    out[b,h,w,f] = sum_k patches[..,k] * dyn[..,k,f]

Algebraic restructuring used here (per pixel p, k=(ki,kj,c), channels c'):
    out[p,f] = sum_{c'} x[p,c'] * M2[(f,c'), p]
    M2[(f,c'), p] = sum_k W4[k, (f,c')] * patches[p,k],   W4[k, f*32+c'] = w_gen[c', k*32+f]

So per core (data-parallel over 24 (b,h)-rows each):
    1. patchesT[k, p] materialized as 9 shifted SBUF copies of the transposed,
       zero-padded input slab (no gather needed: patchesT for shift (ki,kj) is
       just a shifted window of xT_pad).
    2. PE: M2T = W4^T @ patchesT per 128-wide (f,c') tile  (3 K-chunks of 128).
    3. DVE: prodT = M2T * xTrep (x^T replicated 4x over partitions).
    4. PE: out^T[f, p] = sum over (f,c') partitions via 0/1 selection matmuls
       (contract c' groups), accumulated in PSUM.
    5. out^T DMA'd out; host transposes + reassembles.

The bias (b_gen) contribution sum_k patches[p,k]*Bg[k,f] is added with 3 extra
accumulating matmuls only when b_gen is nonzero (it is structurally zero here).
"""

import numpy as np

import concourse.bass as bass
import concourse.mybir as mybir
import concourse.tile as tile
from concourse import bacc
from concourse.bass import ts

F32 = mybir.dt.float32
F32R = mybir.dt.float32r  # relaxed fp32: full-rate PE matmul (fp32 is 4x slower)

B, H, W, C = 2, 96, 96, 32
KS = 3  # conv kernel size
NCORES = 8
RPC = (B * H) // NCORES          # (b,h)-rows per core = 24
PIX = RPC * W                    # real pixels per core = 2304
PADW = W + 2                     # 98
XTCOLS = (RPC + 2) * PADW        # 26*98 = 2548
NSHIFT = KS * KS                 # 9
KDIM = NSHIFT * C                # 288
NFC = C * C                      # 1024 (f,c') pairs
NPASS = 512                      # pixels per matmul pass (PSUM bank / fp32 max)
LAG = 4                          # sel-matmul emission lag (hide DVE latency)


def _build_nc(has_bias: bool, reps: int = 1, mm_dt=F32R, pack_c2: bool | None = None) -> bass.Bass:
    """Build the Bass module.  reps>1 wraps the whole body (input DMAs,
    patchesT build, compute, output DMAs) in a hardware For_i loop — used
    only for benchmarking (amortizes the ~80ms axon dispatch overhead)."""
    if pack_c2 is None:
        import os

        pack_c2 = os.environ.get("KERNEL_PACK_C2", "1") == "1"
    nc = bacc.Bacc("TRN2", target_bir_lowering=False, debug=False)

    xt_d = nc.dram_tensor("xt", [C, XTCOLS], mm_dt, kind="ExternalInput")
    xr_d = nc.dram_tensor("xr", [128, PIX], F32, kind="ExternalInput")
    w4_d = nc.dram_tensor("w4", [128, 2, NFC], mm_dt, kind="ExternalInput")
    w4b_d = nc.dram_tensor("w4b", [128, NFC], mm_dt, kind="ExternalInput")
    sel_d = nc.dram_tensor("sel", [128, NCORES, C], mm_dt, kind="ExternalInput")
    if has_bias:
        bg_d = nc.dram_tensor("bg", [128, 3, C], mm_dt, kind="ExternalInput")
    out_d = nc.dram_tensor("outT", [C, PIX], F32, kind="ExternalOutput")

    passes = []
    p0 = 0
    while p0 < PIX:
        n = min(NPASS, PIX - p0)
        passes.append((p0, n))
        p0 += n

    with tile.TileContext(nc) as tc:
        with (
            tc.tile_pool(name="const", bufs=1) as const_pool,
            tc.tile_pool(name="prod", bufs=5) as prod_pool,
            tc.tile_pool(name="outsb", bufs=2) as out_pool,
            tc.tile_pool(name="psum_m2", bufs=5, space="PSUM") as psum_m2,
            tc.tile_pool(name="psum_out", bufs=2, space="PSUM") as psum_out,
        ):

          def body():
            xt_sb = const_pool.tile([C, XTCOLS], mm_dt, tag="xt")
            w4_sb = const_pool.tile([128, 2, NFC], mm_dt, tag="w4")
            w4b_sb = const_pool.tile([128, NFC], mm_dt, tag="w4b")
            sel_sb = const_pool.tile([128, NCORES, C], mm_dt, tag="sel")
            patchesT = const_pool.tile([128, 3, PIX], mm_dt, tag="patchesT")
            xtrep = const_pool.tile([128, PIX], F32, tag="xtrep")
            if has_bias:
                bg_sb = const_pool.tile([128, 3, C], mm_dt, tag="bg")
                nc.sync.dma_start(bg_sb[:], bg_d[:])

            nc.sync.dma_start(xt_sb[:], xt_d[:])
            nc.sync.dma_start(xtrep[:], xr_d[:])
            nc.sync.dma_start(w4_sb[:], w4_d[:])
            nc.sync.dma_start(w4b_sb[:], w4b_d[:])
            nc.sync.dma_start(sel_sb[:], sel_d[:])

            # padded-slab view [c, row, col] of the transposed input
            xtv = xt_sb[:].rearrange("p (r w) -> p r w", w=PADW)

            # shifted copies build patchesT: shifts 0-7 fill chunks 0-1;
            # shift 8 (k=256..287) is replicated at all 4 partition bases of
            # chunk 2 so its K=32 matmuls can be row-tiled 4-concurrent.
            for s in range(NSHIFT):
                ki, kj = divmod(s, KS)
                chunk, prow = divmod(s, 4)
                reps_dst = [prow] if s < 8 else [0, 1, 2, 3]
                for pr in reps_dst:
                    dst = patchesT[32 * pr : 32 * pr + 32, chunk, :].rearrange(
                        "p (r w) -> p r w", w=W
                    )
                    nc.sync.dma_start(dst, xtv[:, ki : ki + RPC, kj : kj + W])

            pending = []

            def flush_one():
                t, prodT, out_ps, n, px0 = pending.pop(0)
                if t == 0:
                    if has_bias:
                        for c in range(3):
                            nc.tensor.matmul(
                                out_ps[:, :n],
                                bg_sb[:, c, :],
                                patchesT[:, c, px0 : px0 + n],
                                start=(c == 0),
                                stop=False,
                            )
                nc.tensor.matmul(
                    out_ps[:, :n],
                    sel_sb[:, t, :],
                    prodT[:, :n],
                    start=(t == 0 and not has_bias),
                    stop=(t == NCORES - 1),
                )
                if t == NCORES - 1:
                    o_sb = out_pool.tile([C, NPASS], F32, tag="osb")
                    nc.scalar.copy(o_sb[:, :n], out_ps[:, :n])
                    nc.sync.dma_start(out_d[:, px0 : px0 + n], o_sb[:, :n])

            for px0, n in passes:
                out_ps = psum_out.tile([C, NPASS], F32, tag="outps")
                for g in range(2):  # two groups of 4 (f,c')-tiles
                    m2ts = []
                    for u in range(4):
                        t = 4 * g + u
                        m2t = psum_m2.tile([128, NPASS], F32, tag="m2t")
                        m2ts.append(m2t)
                        for c in range(2):
                            nc.tensor.matmul(
                                m2t[:, :n],
                                w4_sb[:, c, ts(t, 128)],
                                patchesT[:, c, px0 : px0 + n],
                                start=(c == 0),
                                stop=False,
                            )
                    # 4 concurrent K=32 row-tiled matmuls finish the 4 tiles
                    for u in range(4):
                        t = 4 * g + u
                        up = u if pack_c2 else 0
                        kw = {"tile_position": (32 * u, 0)} if pack_c2 else {}
                        nc.tensor.matmul(
                            m2ts[u][:, :n],
                            w4b_sb[ts(up, 32), ts(t, 128)],
                            patchesT[ts(up, 32), 2, px0 : px0 + n],
                            start=False,
                            stop=True,
                            **kw,
                        )
                    for u in range(4):
                        t = 4 * g + u
                        prodT = prod_pool.tile([128, NPASS], mm_dt, tag="prodT")
                        nc.vector.tensor_tensor(
                            prodT[:, :n],
                            m2ts[u][:, :n],
                            xtrep[:, px0 : px0 + n],
                            op=mybir.AluOpType.mult,
                        )
                        pending.append((t, prodT, out_ps, n, px0))
                    while len(pending) > LAG:
                        flush_one()
            while pending:
                flush_one()

          if reps == 1:
              body()
          else:
              with tc.For_i(0, reps, 1):
                  body()

    nc.compile()
    return nc


def _host_prep(x, w_gen, b_gen):
    """Build per-core input maps (host-side sharding / layout only)."""
    x = np.ascontiguousarray(x, dtype=np.float32)
    w_gen = np.ascontiguousarray(w_gen, dtype=np.float32)
    b_gen = np.ascontiguousarray(b_gen, dtype=np.float32)

    has_bias = bool(np.any(b_gen))

    # W4[k, f*32+c'] = w_gen[c', k*32+f]; pad K=288 into 3 chunks of 128
    w4 = np.transpose(w_gen.reshape(C, KDIM, C), (1, 2, 0)).reshape(KDIM, NFC)
    w4p = np.zeros((128, 2, NFC), np.float32)
    w4p[:, 0, :] = w4[0:128]
    w4p[:, 1, :] = w4[128:256]
    # chunk-2 weights (k=256..287) replicated at all four partition bases so
    # the K=32 matmuls can be row-tiled at tile_position (32u, 0) for any u
    w4b = np.zeros((128, NFC), np.float32)
    for u in range(4):
        w4b[32 * u : 32 * u + 32, :] = w4[256:KDIM, :]

    # selection matrices: sel[p, t, f] = 1 iff f == 4t + p//32
    sel = np.zeros((128, NCORES, C), np.float32)
    for t in range(NCORES):
        for p in range(128):
            sel[p, t, 4 * t + p // 32] = 1.0

    if has_bias:
        bg = b_gen.reshape(KDIM, C)
        bgp = np.zeros((128, 3, C), np.float32)
        bgp[:, 0, :] = bg[0:128]
        bgp[:, 1, :] = bg[128:256]
        bgp[:C, 2, :] = bg[256:KDIM]

    xpad = np.zeros((B, H + 2, W + 2, C), np.float32)
    xpad[:, 1 : H + 1, 1 : W + 1, :] = x

    in_maps = []
    for core in range(NCORES):
        b, j = divmod(core, NCORES // B)
        slab = xpad[b, RPC * j : RPC * j + RPC + 2]           # (26, 98, 32)
        xt = np.ascontiguousarray(slab.reshape(XTCOLS, C).T)  # (32, 2548)
        # x^T at real pixels, replicated 4x over partitions (TT multiplier)
        xc = x[b, RPC * j : RPC * j + RPC].reshape(PIX, C).T  # (32, 2304)
        xr = np.ascontiguousarray(np.tile(xc, (4, 1)))        # (128, 2304)
        m = {"xt": xt, "xr": xr, "w4": w4p, "w4b": w4b, "sel": sel}
        if has_bias:
            m["bg"] = bgp
        in_maps.append(m)
    return in_maps, has_bias


def _assemble(outs):
    out = np.empty((B, H, W, C), np.float32)
    for core in range(NCORES):
        b, j = divmod(core, NCORES // B)
        out[b, RPC * j : RPC * j + RPC] = (
            outs[core]["outT"].T.reshape(RPC, W, C)
        )
    return out


# ---------------------------------------------------------------------------
# Execution: persistent jitted shard_map callable (axon/PJRT path), so that
# repeated kernel() calls don't re-trace.  Mirrors bass2jax.run_bass_via_pjrt.
# ---------------------------------------------------------------------------
_RUNNERS = {}


def _get_runner(has_bias: bool, reps: int = 1):
    key = (has_bias, reps)
    if key in _RUNNERS:
        return _RUNNERS[key]

    import jax
    from jax.experimental.shard_map import shard_map
    from jax.sharding import Mesh, NamedSharding, PartitionSpec

    from concourse import bass2jax

    nc = _build_nc(has_bias, reps)
    bass2jax.install_neuronx_cc_hook()

    partition_name = (
        nc.partition_id_tensor.name if nc.partition_id_tensor else None
    )
    in_names, out_names, out_avals, zero_shapes = [], [], [], []
    for alloc in nc.m.functions[0].allocations:
        if not isinstance(alloc, mybir.MemoryLocationSet):
            continue
        name = alloc.memorylocations[0].name
        if alloc.kind == "ExternalInput":
            if name != partition_name:
                in_names.append(name)
        elif alloc.kind == "ExternalOutput":
            out_names.append(name)
            shape = tuple(alloc.tensor_shape)
            dtype = mybir.dt.np(alloc.dtype)
            out_avals.append(jax.core.ShapedArray(shape, dtype))
            zero_shapes.append((shape, dtype))
    n_params = len(in_names)
    n_outs = len(out_names)
    all_in_names = in_names + out_names
    if partition_name is not None:
        all_in_names = all_in_names + [partition_name]

    def _body(*args):
        operands = list(args)
        if partition_name is not None:
            operands.append(bass2jax.partition_id_tensor())
        outs = bass2jax._bass_exec_p.bind(
            *operands,
            out_avals=tuple(out_avals),
            in_names=tuple(all_in_names),
            out_names=tuple(out_names),
            lowering_input_output_aliases=(),
            sim_require_finite=True,
            sim_require_nnan=True,
            nc=nc,
        )
        return tuple(outs)

    devices = jax.devices()[:NCORES]
    assert len(devices) == NCORES, f"need {NCORES} cores, got {len(devices)}"
    mesh = Mesh(np.asarray(devices), ("core",))
    donate = tuple(range(n_params, n_params + n_outs))
    sharded = jax.jit(
        shard_map(
            _body,
            mesh=mesh,
            in_specs=(PartitionSpec("core"),) * (n_params + n_outs),
            out_specs=(PartitionSpec("core"),) * n_outs,
            check_rep=False,
        ),
        donate_argnums=donate,
        keep_unused=True,
    )

    def run(in_maps):
        concat_in = [
            np.concatenate([np.asarray(m[name]) for m in in_maps], axis=0)
            for name in in_names
        ]
        concat_zeros = [
            np.zeros((NCORES * s[0], *s[1:]), dt) for (s, dt) in zero_shapes
        ]
        out_arrs = sharded(*concat_in, *concat_zeros)
        out_arrs = [np.asarray(a) for a in jax.block_until_ready(out_arrs)]
        return [
            {
                name: out_arrs[i].reshape(NCORES, *out_avals[i].shape)[c]
                for i, name in enumerate(out_names)
            }
            for c in range(NCORES)
        ]

    def time_resident(in_maps, n_calls=12):
        """Min wall time per dispatch with device-resident operands and no
        output download (measures dispatch overhead + on-device exec)."""
        import time as _time

        sh = NamedSharding(mesh, PartitionSpec("core"))
        concat_in = [
            jax.device_put(
                np.concatenate([np.asarray(m[name]) for m in in_maps], axis=0), sh
            )
            for name in in_names
        ]
        concat_zeros = [
            jax.device_put(np.zeros((NCORES * s[0], *s[1:]), dt), sh)
            for (s, dt) in zero_shapes
        ]
        jax.block_until_ready(concat_in)
        jax.block_until_ready(concat_zeros)
        nodonate = jax.jit(
            shard_map(
                _body,
                mesh=mesh,
                in_specs=(PartitionSpec("core"),) * (n_params + n_outs),
                out_specs=(PartitionSpec("core"),) * n_outs,
                check_rep=False,
            ),
            keep_unused=True,
        )
        jax.block_until_ready(nodonate(*concat_in, *concat_zeros))  # compile
        best = float("inf")
        for _ in range(n_calls):
            t0 = _time.perf_counter()
            jax.block_until_ready(nodonate(*concat_in, *concat_zeros))
            best = min(best, _time.perf_counter() - t0)
        return best

    run.time_resident = time_resident
    _RUNNERS[key] = run
    return run


def kernel(x, w_gen, b_gen):
    in_maps, has_bias = _host_prep(x, w_gen, b_gen)
    run = _get_runner(has_bias)
    outs = run(in_maps)
    return _assemble(outs)


BENCH_REPS = 64


def benchmark(x, w_gen, b_gen, iters=12):
    """Estimate per-execution device time: run the kernel body BENCH_REPS
    times inside one NEFF (hardware For_i around the whole body) and take the
    difference quotient against the single-rep NEFF.  This cancels the ~80ms
    axon dispatch overhead and the NEFF load/launch constants."""
    in_maps, has_bias = _host_prep(x, w_gen, b_gen)
    run1 = _get_runner(has_bias, 1)
    runN = _get_runner(has_bias, BENCH_REPS)
    t1 = run1.time_resident(in_maps, iters)
    tn = runN.time_resident(in_maps, iters)
    per_exec = (tn - t1) / (BENCH_REPS - 1)
    print(
        f"[benchmark] plain={t1 * 1e3:.2f} ms  loop{BENCH_REPS}={tn * 1e3:.2f} ms"
        f"  -> per-exec {per_exec * 1e6:.1f} us"
    )
    return per_exec * 1e9
